# revision 11
# baseline (speedup 1.0000x reference)
"""Trainium2 Bass kernel for GQA attention (B=2, T=4096, D=2048, N=8 q-heads,
K=1 kv-head, H=256) with RoPE + causal mask + output projection.

Sharding: data-parallel on batch (2) x tensor-parallel on query heads
(4 groups of 2 heads) = 8 cores. Each core computes a partial output
y_c = sum_{n in its 2 heads} softmax(q_n k^T) v @ out_w[n] for its batch;
the host sums the 4 partials per batch.

The device kernel is identical on every core (single NEFF, SPMD); per-core
behaviour comes only from per-core input data:
  xt   [2048, 4096] bf16 : x[b]^T  (pre-transposed + bf16 on host)
  qw   [2, 2048, 256] bf16 : q_w for the core's 2 heads, pre-scaled by H^-0.5
  kvw  [2, 2048, 256] bf16 : k/v projection weights (shared kv head)
  outw [2, 256, 2048] bf16 : out_w for the core's 2 heads
  cost/sint [128, 4096] f32 : RoPE cos/sin tables (timescale j x position t)
Output: y [4096, 2048] f32 partial.

Flash-attention layout: everything transposed (S^T = K^T^T-contraction) so
softmax statistics land in matmuls:
  K^T,Q^T [h, t] from projections directly; logits S^T [s-chunk 128, t 512]
  in PSUM; exp on ACT -> P^T bf16; PV as V^T-stationary matmul giving
  O^T [h-half, t] accumulated over s-chunks in PSUM; denominator via
  ones-column matmul; normalization by a PE-broadcast reciprocal.
"""

import os

import numpy as np
import ml_dtypes

B, T, D, N, H = 2, 4096, 2048, 8, 256
NCORES = 8
HH = H // 2  # 128, also the RoPE pair offset and partition size
TQ = 512     # query-tile columns (moving dim of logits matmul)
NT = T // TQ # 8 query tiles
NDC = D // 128  # 16 contraction chunks over D

_CACHE = {}
LAST_RESULT = None  # BassKernelResults of the most recent device run (for test harness)


def _split_excess_waits(nc):
    """The walrus in this container accepts at most 1 sync-wait per
    instruction (2 for EventSemaphore); Tile attaches one wait per producer
    semaphore. Hoist excess waits onto injected same-engine NOPs immediately
    before the instruction (engine queues are in-order, so waiting A then B
    sequentially == waiting {A,B} at once)."""
    import bass_rust
    import concourse.mybir as mybir

    n_split = 0
    for f in nc.m.functions:
        for bb in f.blocks:
            insts = bb.instructions
            out = []
            changed = False
            for inst in insts:
                si = inst.sync_info
                waits = list(si.on_wait) if si is not None and si.on_wait else []
                cap = 2 if isinstance(inst, mybir.InstEventSemaphore) else 1
                if len(waits) > cap:
                    changed = True
                    for w in waits[:-cap]:
                        nop = mybir.InstNoOp(
                            name=f"waitsplit_{n_split}", ins=[], outs=[]
                        )
                        n_split += 1
                        nop.engine = inst.engine
                        nop.sync_info = bass_rust.SyncInfo(on_wait=[w], on_update=[])
                        out.append(nop)
                    inst.sync_info = bass_rust.SyncInfo(
                        on_wait=waits[-cap:], on_update=si.on_update
                    )
                out.append(inst)
            if changed:
                insts[:] = out
                if bb.instructions[0].name != out[0].name or len(bb.instructions) != len(out):
                    raise RuntimeError("basic block instruction list not live-mutable")
    return n_split


def _rope_pair(nc, tmps, p0, p1, cos_s, sin_s, out0, out1):
    """out0 = p0*cos - p1*sin ; out1 = p1*cos + p0*sin  (RoPE half-pair).
    p0/p1: [128, L] f32 PSUM; copied to SBUF first (frees the PSUM bank
    after ~1 ACT op instead of after 4 DVE ops). cos/sin: [128, L] f32
    SBUF, out0/out1: [128, L] bf16 SBUF."""
    import concourse.mybir as mybir

    L = p0.shape[-1]
    c0 = tmps.tile([128, TQ], mybir.dt.float32, tag="projc")
    c1 = tmps.tile([128, TQ], mybir.dt.float32, tag="projc")
    nc.scalar.copy(c0[:, :L], p0)
    nc.vector.tensor_copy(c1[:, :L], p1)
    t0 = tmps.tile([128, TQ], mybir.dt.float32, tag="ropetmp")
    t1 = tmps.tile([128, TQ], mybir.dt.float32, tag="ropetmp")
    nc.vector.tensor_mul(t0[:, :L], c0[:, :L], cos_s)
    nc.vector.tensor_mul(t1[:, :L], c1[:, :L], sin_s)
    nc.vector.tensor_sub(out0, t0[:, :L], t1[:, :L])
    t2 = tmps.tile([128, TQ], mybir.dt.float32, tag="ropetmp")
    t3 = tmps.tile([128, TQ], mybir.dt.float32, tag="ropetmp")
    nc.vector.tensor_mul(t2[:, :L], c1[:, :L], cos_s)
    nc.vector.tensor_mul(t3[:, :L], c0[:, :L], sin_s)
    nc.vector.tensor_add(out1, t2[:, :L], t3[:, :L])


def _build_nc():
    import concourse.bass as bass
    import concourse.mybir as mybir
    import concourse.tile as tile

    F32 = mybir.dt.float32
    BF16 = mybir.dt.bfloat16
    EXP = mybir.ActivationFunctionType.Exp

    nc = bass.Bass()
    xt = nc.dram_tensor("xt", [D, T], BF16, kind="ExternalInput")
    qw = nc.dram_tensor("qw", [2, D, H], BF16, kind="ExternalInput")
    kvw = nc.dram_tensor("kvw", [2, D, H], BF16, kind="ExternalInput")
    outw = nc.dram_tensor("outw", [2, H, D], BF16, kind="ExternalInput")
    cost = nc.dram_tensor("cost", [HH, T], F32, kind="ExternalInput")
    sint = nc.dram_tensor("sint", [HH, T], F32, kind="ExternalInput")
    y = nc.dram_tensor("y", [T, D], F32, kind="ExternalOutput")

    with tile.TileContext(nc) as tc:
        with (
            tc.tile_pool(name="const", bufs=1) as constp,
            tc.tile_pool(name="persist", bufs=1) as persist,
            tc.tile_pool(name="stream", bufs=2) as stream,
            tc.tile_pool(name="qtp", bufs=4) as qtp,
            tc.tile_pool(name="tmps", bufs=3) as tmps,
            tc.tile_pool(name="ptp", bufs=3) as ptp,
            tc.tile_pool(name="otp", bufs=2) as otp,
            tc.tile_pool(name="ysp", bufs=1) as ysp,
            tc.tile_pool(name="psum", bufs=1, space="PSUM") as psum,
        ):
            # --- constants -------------------------------------------------
            # Causal staircase: cmask[p, c] = 0 if c >= p else -1e30.
            cmask = constp.tile([128, TQ], F32)
            nc.gpsimd.memset(cmask, 0.0)
            nc.gpsimd.affine_select(
                out=cmask,
                in_=cmask,
                compare_op=mybir.AluOpType.is_ge,
                fill=-1.0e30,
                base=0,
                pattern=[[1, TQ]],
                channel_multiplier=-1,
            )
            ones_col = constp.tile([128, 1], BF16)
            nc.vector.memset(ones_col, 1.0)
            ones_row = constp.tile([1, 128], F32)
            nc.vector.memset(ones_row, 1.0)

            # --- resident weights / tables --------------------------------
            # DMA order matters for startup latency: K-projection needs kvs
            # (+ the first x^T slice) first; ows is only needed ~60us in.
            kvs = constp.tile([128, 2, NDC, H], BF16)
            nc.sync.dma_start(out=kvs, in_=kvw.rearrange("n (c p) h -> p n c h", p=128))
            coss = constp.tile([128, T], F32)
            nc.sync.dma_start(out=coss, in_=cost[:, :])
            sins = constp.tile([128, T], F32)
            nc.sync.dma_start(out=sins, in_=sint[:, :])
            qws = constp.tile([128, 2, NDC, H], BF16)
            nc.sync.dma_start(out=qws, in_=qw.rearrange("n (c p) h -> p n c h", p=128))
            ows = constp.tile([128, 2, 2, D], BF16)
            nc.sync.dma_start(
                out=ows, in_=outw.rearrange("n (hh p) d -> p n hh d", p=128)
            )

            # K^T halves [h-half, t] and V chunks [s-in-chunk, h], grown per tile.
            kts = persist.tile([128, 2, T], BF16)
            vs = persist.tile([128, T // 128, H], BF16)

            xt_r = xt.rearrange("(c p) t -> p c t", p=128)

            def out_proj(i, ots):
                """Output projection for query tile i (both heads' ot ready)."""
                for ts in range(4):
                    ys = ysp.tile([128, D], F32, tag="ys")
                    ssl = slice(ts * 128, (ts + 1) * 128)
                    for dc in range(4):
                        py = psum.tile([128, 512], F32, tag="work", bufs=3)
                        mm = 0
                        for n in (0, 1):
                            for hh in (0, 1):
                                nc.tensor.matmul(
                                    py,
                                    lhsT=ots[n][:, hh, ssl],
                                    rhs=ows[:, n, hh, dc * 512 : (dc + 1) * 512],
                                    start=(mm == 0),
                                    stop=(mm == 3),
                                )
                                mm += 1
                        nc.scalar.copy(ys[:, dc * 512 : (dc + 1) * 512], py)
                    nc.sync.dma_start(
                        out=y[i * TQ + ts * 128 : i * TQ + (ts + 1) * 128, :], in_=ys
                    )

            pending = None  # (i, ots) whose output projection is deferred
            for i in range(NT):
                tsl = slice(i * TQ, (i + 1) * TQ)
                cos_sl = coss[:, tsl]
                sin_sl = sins[:, tsl]

                # ---- load x^T slice [128, 16, 512] (4 DMAs so the first
                # projection matmuls can start on a quarter of the data) ----
                xts = stream.tile([128, NDC, TQ], BF16, tag="xts")
                for dg in range(4):
                    nc.sync.dma_start(
                        out=xts[:, 4 * dg : 4 * (dg + 1), :],
                        in_=xt_r[:, 4 * dg : 4 * (dg + 1), tsl],
                    )

                # ---- K^T projection + RoPE -------------------------------
                kp0 = psum.tile([128, TQ], F32, tag="projqk", bufs=2)
                kp1 = psum.tile([128, TQ], F32, tag="projqk", bufs=2)
                for hh, kp in ((0, kp0), (1, kp1)):
                    for d in range(NDC):
                        nc.tensor.matmul(
                            kp,
                            lhsT=kvs[:, 0, d, hh * 128 : (hh + 1) * 128],
                            rhs=xts[:, d, :],
                            start=(d == 0),
                            stop=(d == NDC - 1),
                        )
                _rope_pair(
                    nc, tmps, kp0, kp1, cos_sl, sin_sl,
                    kts[:, 0, tsl], kts[:, 1, tsl],
                )

                # ---- Q^T projections + RoPE (2 heads) --------------------
                qt = []
                for n in (0, 1):
                    qp0 = psum.tile([128, TQ], F32, tag="projqk", bufs=2)
                    qp1 = psum.tile([128, TQ], F32, tag="projqk", bufs=2)
                    for hh, qp in ((0, qp0), (1, qp1)):
                        for d in range(NDC):
                            nc.tensor.matmul(
                                qp,
                                lhsT=qws[:, n, d, hh * 128 : (hh + 1) * 128],
                                rhs=xts[:, d, :],
                                start=(d == 0),
                                stop=(d == NDC - 1),
                            )
                    qtn = qtp.tile([128, 2, TQ], BF16, tag="qt")
                    _rope_pair(
                        nc, tmps, qp0, qp1, cos_sl, sin_sl,
                        qtn[:, 0, :], qtn[:, 1, :],
                    )
                    qt.append(qtn)

                # ---- V projection ----------------------------------------
                for ts in range(4):
                    vp = psum.tile([128, H], F32, tag="projqk", bufs=2)
                    for d in range(NDC):
                        nc.tensor.matmul(
                            vp,
                            lhsT=xts[:, d, ts * 128 : (ts + 1) * 128],
                            rhs=kvs[:, 1, d, :],
                            start=(d == 0),
                            stop=(d == NDC - 1),
                        )
                    nc.vector.tensor_copy(vs[:, 4 * i + ts, :], vp)

                # ---- deferred output projection of the previous tile -----
                # (emitted here so PE has this tile's projection work to chew
                # while the previous tile's normalize chain resolves)
                if pending is not None:
                    out_proj(*pending)
                    pending = None

                # ---- attention for each head -----------------------------
                ots = []
                nchunks = 4 * i + 4
                for n in (0, 1):
                    po0 = psum.tile([128, TQ], F32, tag="po0")
                    po1 = psum.tile([128, TQ], F32, tag="po1")
                    pd = psum.tile([1, TQ], F32, tag="pd")
                    for k in range(nchunks):
                        col0 = max(0, k - 4 * i) * 128
                        nn_ = TQ - col0
                        ksl = slice(k * 128, (k + 1) * 128)
                        pl = psum.tile([128, TQ], F32, tag="work", bufs=3)
                        nc.tensor.matmul(
                            pl[:, col0:],
                            lhsT=kts[:, 0, ksl],
                            rhs=qt[n][:, 0, col0:],
                            start=True,
                            stop=False,
                        )
                        nc.tensor.matmul(
                            pl[:, col0:],
                            lhsT=kts[:, 1, ksl],
                            rhs=qt[n][:, 1, col0:],
                            start=False,
                            stop=True,
                        )
                        if k >= 4 * i:
                            nc.vector.tensor_add(
                                pl[:, col0:], pl[:, col0:], cmask[:, :nn_]
                            )
                        pt = ptp.tile([128, TQ], BF16, tag="pt")
                        nc.scalar.activation(pt[:, col0:], pl[:, col0:], EXP)
                        first, last = (k == 0), (k == nchunks - 1)
                        for hh, po in ((0, po0), (1, po1)):
                            nc.tensor.matmul(
                                po[:, col0:],
                                lhsT=vs[:, k, hh * 128 : (hh + 1) * 128],
                                rhs=pt[:, col0:],
                                start=first,
                                stop=last,
                            )
                        nc.tensor.matmul(
                            pd[:, col0:],
                            lhsT=ones_col,
                            rhs=pt[:, col0:],
                            start=first,
                            stop=last,
                        )

                    # Eagerly evacuate PSUM accumulators to SBUF so the po/pd
                    # banks free up for the next head while the (slow-ish)
                    # reciprocal chain runs off the PE critical path.
                    pos = tmps.tile([128, 2, TQ], F32, tag="pos", bufs=2)
                    nc.scalar.copy(pos[:, 0, :], po0)
                    nc.scalar.copy(pos[:, 1, :], po1)
                    pds = tmps.tile([1, TQ], F32, tag="pds", bufs=2)
                    nc.scalar.copy(pds, pd)
                    # normalize: O^T * (1/denom) broadcast along partitions
                    rd = tmps.tile([1, TQ], F32, tag="rd", bufs=2)
                    nc.vector.reciprocal(rd, pds)
                    pb = psum.tile([128, TQ], F32, tag="work", bufs=3)
                    nc.tensor.matmul(pb, lhsT=ones_row, rhs=rd, start=True, stop=True)
                    ot = otp.tile([128, 2, TQ], BF16, tag=f"ot{n}")
                    nc.vector.tensor_mul(ot[:, 0, :], pos[:, 0, :], pb)
                    nc.vector.tensor_mul(ot[:, 1, :], pos[:, 1, :], pb)
                    ots.append(ot)

                pending = (i, ots)
            out_proj(*pending)
    n = _split_excess_waits(nc)
    print(f"kernel build: split {n} excess waits")
    return nc


def _is_causal(mask):
    """mask: [B, T, T] bool — check it's exactly the causal tril mask."""
    tri = np.tril(np.ones((T, T), dtype=bool))
    return all(np.array_equal(mask[b], tri) for b in range(mask.shape[0]))


def _numpy_reference(x, segment_pos, attn_mask, q_w, kv_w, out_w):
    """Slow exact fallback for non-causal masks (matches reference.py)."""
    x = np.asarray(x, np.float32)
    out = np.zeros((B, T, D), np.float32)
    j = np.arange(HH, dtype=np.float32)
    timescale = 10000.0 ** (2.0 * j / H)
    for b in range(B):
        ang = segment_pos[b][:, None].astype(np.float32) / timescale[None, :]
        cos, sin = np.cos(ang), np.sin(ang)  # [T, 128]
        k = x[b] @ kv_w[0, 0]  # [T, H]
        v = x[b] @ kv_w[1, 0]
        k = np.concatenate(
            [k[:, :HH] * cos - k[:, HH:] * sin, k[:, HH:] * cos + k[:, :HH] * sin], 1
        )
        for n in range(N):
            q = x[b] @ q_w[n]
            q = np.concatenate(
                [q[:, :HH] * cos - q[:, HH:] * sin, q[:, HH:] * cos + q[:, :HH] * sin],
                1,
            ) * (H ** -0.5)
            logits = q @ k.T  # [T, T]
            logits = np.where(attn_mask[b], logits, -2.3819763e38)
            logits -= logits.max(-1, keepdims=True)
            p = np.exp(logits)
            p /= p.sum(-1, keepdims=True)
            out[b] += (p.astype(np.float32) @ v) @ out_w[n]
    return out


def kernel(x, segment_pos, attn_mask, q_w, kv_w, out_w):
    global LAST_RESULT
    x = np.asarray(x)
    segment_pos = np.asarray(segment_pos)
    attn_mask = np.asarray(attn_mask)
    q_w = np.asarray(q_w)
    kv_w = np.asarray(kv_w)
    out_w = np.asarray(out_w)
    assert x.shape == (B, T, D) and q_w.shape == (N, D, H)

    if not _is_causal(attn_mask):
        return _numpy_reference(x, segment_pos, attn_mask, q_w, kv_w, out_w)

    from concourse.bass_utils import run_bass_kernel_spmd

    if "nc" not in _CACHE:
        _CACHE["nc"] = _build_nc()
    nc = _CACHE["nc"]

    bf16 = ml_dtypes.bfloat16
    # Per-batch host prep
    xts, coss, sins = [], [], []
    j = np.arange(HH, dtype=np.float32)
    timescale = 10000.0 ** (2.0 * j / H)
    for b in range(B):
        xts.append(np.ascontiguousarray(x[b].T).astype(bf16))
        ang = segment_pos[b][None, :].astype(np.float32) / timescale[:, None]
        coss.append(np.cos(ang).astype(np.float32))
        sins.append(np.sin(ang).astype(np.float32))
    kvw_host = np.ascontiguousarray(kv_w[:, 0]).astype(bf16)  # [2, D, H]
    qw_scaled = (q_w * np.float32(H ** -0.5)).astype(bf16)  # [N, D, H]
    outw_host = out_w.astype(bf16)  # [N, H, D]

    in_maps = []
    for c in range(NCORES):
        b, m = c // 4, c % 4
        in_maps.append(
            {
                "xt": xts[b],
                "qw": np.ascontiguousarray(qw_scaled[2 * m : 2 * m + 2]),
                "kvw": kvw_host,
                "outw": np.ascontiguousarray(outw_host[2 * m : 2 * m + 2]),
                "cost": coss[b],
                "sint": sins[b],
            }
        )

    trace = bool(int(os.environ.get("KERNEL_TRACE", "0")))
    res = run_bass_kernel_spmd(nc, in_maps, core_ids=list(range(NCORES)), trace=trace)
    LAST_RESULT = res

    out = np.zeros((B, T, D), np.float32)
    for c in range(NCORES):
        out[c // 4] += res.results[c]["y"]
    return out
